# revision 5
# baseline (speedup 1.0000x reference)
"""Trainium2 Bass kernel for the block-diagonal grouped linear
(e3nn-style per-l channel mixing):

    out[:, l^2:l^2+2l+1, :] = path_weights[l] * x[:, l^2:..., :] @ weights[l]

Strategy: data-parallel over the node axis (8 cores x 6250 nodes).
On the host we slice each core's shard into the 4 l-blocks and transpose
each to [c_in=128, rows_l] layout.  On device, each l-block is a plain
matrix product with weights[l] stationary on the PE array:

    outT[l] = (pw_l * W_l^T) @ xT[l]      (psum = lhsT.T @ rhs, lhsT = W_l)

so the kernel is a pure stream: DMA-in 1MB chunks -> one fp32r matmul per
512 columns -> VectorE scale-by-pw copy PSUM->SBUF -> DMA-out.  No
on-device transposes; both DMA directions move contiguous-per-partition
chunks.  fp32r runs the PE at full rate for moving free dim >= 256.
"""

import sys
import types

if "/opt/trn_rl_repo" not in sys.path:
    sys.path.insert(0, "/opt/trn_rl_repo")

import numpy as np

N_CORES = 8
N_NODES = 50000
LMAX = 3
CH = 128
NPC = N_NODES // N_CORES  # nodes per core
ROWS = [NPC * (2 * l + 1) for l in range(LMAX + 1)]  # rows per l per core
CHUNK = 2048  # columns per DMA chunk (1 MiB fp32 at 128 partitions)
MM = 512  # moving free dim per matmul (one PSUM bank fp32)

_nc = None  # compiled Bass program, cached across kernel() calls
LAST_RESULTS = None  # BassKernelResults of the last run (for test harnesses)


def _install_ntff_hook():
    """Make trace=True work under axon: register the NTFF profile hook the
    image's antenv package is missing.  Harmless if anything is absent."""
    try:
        import antenv

        if "antenv.axon_hooks" in sys.modules:
            return
        mod = types.ModuleType("antenv.axon_hooks")
        mod._hook = None

        def set_axon_ntff_profile_hook(h):
            mod._hook = h

        def get_axon_ntff_profile_hook():
            return mod._hook

        mod.set_axon_ntff_profile_hook = set_axon_ntff_profile_hook
        mod.get_axon_ntff_profile_hook = get_axon_ntff_profile_hook
        sys.modules["antenv.axon_hooks"] = mod
        antenv.axon_hooks = mod

        from trn_agent_boot.trn_boot import _ntff_profile_via_ctypes

        hook = _ntff_profile_via_ctypes("/opt/axon/libaxon_pjrt.so")
        if hook is not None:
            set_axon_ntff_profile_hook(hook)
    except Exception:
        pass


def _build():
    import concourse.bacc as bacc
    import concourse.mybir as mybir
    import concourse.tile as tile

    f32 = mybir.dt.float32
    f32r = mybir.dt.float32r

    nc = bacc.Bacc(
        "TRN2", target_bir_lowering=False, debug=False, num_devices=N_CORES
    )

    xt = [
        nc.dram_tensor(f"xt{l}", [CH, ROWS[l]], f32r, kind="ExternalInput").ap()
        for l in range(LMAX + 1)
    ]
    w = nc.dram_tensor("w", [(LMAX + 1) * CH, CH], f32r, kind="ExternalInput").ap()
    pwb = nc.dram_tensor("pwb", [CH, LMAX + 1], f32, kind="ExternalInput").ap()
    outT = [
        nc.dram_tensor(f"outT{l}", [CH, ROWS[l]], f32, kind="ExternalOutput").ap()
        for l in range(LMAX + 1)
    ]

    with tile.TileContext(nc) as tc:
        with (
            tc.tile_pool(name="const", bufs=1) as cpool,
            tc.tile_pool(name="io", bufs=6) as iopool,
            tc.tile_pool(name="psum", bufs=4, space="PSUM") as pspool,
        ):
            w_sb = cpool.tile([CH, LMAX + 1, CH], f32r)
            for l in range(LMAX + 1):
                nc.sync.dma_start(w_sb[:, l, :], w[l * CH : (l + 1) * CH, :])
            pw_sb = cpool.tile([CH, LMAX + 1], f32)
            nc.sync.dma_start(pw_sb[:, :], pwb[:, :])

            for l in range(LMAX + 1):
                for j0 in range(0, ROWS[l], CHUNK):
                    cw = min(CHUNK, ROWS[l] - j0)
                    xt_sb = iopool.tile([CH, CHUNK], f32r, tag="xt")
                    nc.sync.dma_start(xt_sb[:, :cw], xt[l][:, j0 : j0 + cw])
                    out_sb = iopool.tile([CH, CHUNK], f32, tag="out")
                    for k0 in range(0, cw, MM):
                        n = min(MM, cw - k0)
                        ps = pspool.tile([CH, MM], f32)
                        nc.tensor.matmul(
                            ps[:, :n],
                            w_sb[:, l, :],
                            xt_sb[:, k0 : k0 + n],
                            start=True,
                            stop=True,
                        )
                        nc.vector.tensor_scalar_mul(
                            out_sb[:, k0 : k0 + n], ps[:, :n], pw_sb[:, l : l + 1]
                        )
                    # Stores on the ACT HWDGE ring (separate logical DMA queue
                    # from the SP ring carrying the loads).
                    nc.scalar.dma_start(outT[l][:, j0 : j0 + cw], out_sb[:, :cw])

    nc.compile()
    return nc


def kernel(x, weights, path_weights):
    global _nc, LAST_RESULTS
    _install_ntff_hook()
    from concourse.bass_utils import run_bass_kernel_spmd

    if _nc is None:
        _nc = _build()

    x = np.asarray(x, dtype=np.float32)
    weights = np.asarray(weights, dtype=np.float32)
    path_weights = np.asarray(path_weights, dtype=np.float32)

    w_flat = np.ascontiguousarray(weights.reshape((LMAX + 1) * CH, CH))
    pwb = np.ascontiguousarray(
        np.broadcast_to(path_weights[None, :], (CH, LMAX + 1)), dtype=np.float32
    )

    in_maps = []
    for c in range(N_CORES):
        xc = x[c * NPC : (c + 1) * NPC]  # [NPC, 16, CH]
        m = {"w": w_flat, "pwb": pwb}
        for l in range(LMAX + 1):
            s, wd = l * l, 2 * l + 1
            m[f"xt{l}"] = np.ascontiguousarray(
                xc[:, s : s + wd, :].reshape(NPC * wd, CH).T
            )
        in_maps.append(m)

    res = run_bass_kernel_spmd(_nc, in_maps, core_ids=list(range(N_CORES)))
    LAST_RESULTS = res

    out = np.empty((N_NODES, (LMAX + 1) ** 2, CH), dtype=np.float32)
    for c in range(N_CORES):
        for l in range(LMAX + 1):
            s, wd = l * l, 2 * l + 1
            out[c * NPC : (c + 1) * NPC, s : s + wd, :] = (
                res.results[c][f"outT{l}"].T.reshape(NPC, wd, CH)
            )
    return out


# revision 6
# speedup vs baseline: 1.0231x; 1.0231x over previous
"""Trainium2 Bass kernel for the block-diagonal grouped linear
(e3nn-style per-l channel mixing):

    out[:, l^2:l^2+2l+1, :] = path_weights[l] * x[:, l^2:..., :] @ weights[l]

Strategy: data-parallel over the node axis (8 cores x 6250 nodes).
On the host we slice each core's shard into the 4 l-blocks and transpose
each to [c_in=128, rows_l] layout.  On device, each l-block is a plain
matrix product with weights[l] stationary on the PE array:

    outT[l] = (pw_l * W_l^T) @ xT[l]      (psum = lhsT.T @ rhs, lhsT = W_l)

so the kernel is a pure stream: DMA-in 1MB chunks -> one fp32r matmul per
512 columns -> VectorE scale-by-pw copy PSUM->SBUF -> DMA-out.  No
on-device transposes; both DMA directions move contiguous-per-partition
chunks.  fp32r runs the PE at full rate for moving free dim >= 256.
"""

import sys
import types

if "/opt/trn_rl_repo" not in sys.path:
    sys.path.insert(0, "/opt/trn_rl_repo")

import numpy as np

N_CORES = 8
N_NODES = 50000
LMAX = 3
CH = 128
NPC = N_NODES // N_CORES  # nodes per core
ROWS = [NPC * (2 * l + 1) for l in range(LMAX + 1)]  # rows per l per core
CHUNK = 2048  # columns per DMA chunk (1 MiB fp32 at 128 partitions)
MM = 512  # moving free dim per matmul (one PSUM bank fp32)

_nc = None  # compiled Bass program, cached across kernel() calls
LAST_RESULTS = None  # BassKernelResults of the last run (for test harnesses)


def _install_ntff_hook():
    """Make trace=True work under axon: register the NTFF profile hook the
    image's antenv package is missing.  Harmless if anything is absent."""
    try:
        import antenv

        if "antenv.axon_hooks" in sys.modules:
            return
        mod = types.ModuleType("antenv.axon_hooks")
        mod._hook = None

        def set_axon_ntff_profile_hook(h):
            mod._hook = h

        def get_axon_ntff_profile_hook():
            return mod._hook

        mod.set_axon_ntff_profile_hook = set_axon_ntff_profile_hook
        mod.get_axon_ntff_profile_hook = get_axon_ntff_profile_hook
        sys.modules["antenv.axon_hooks"] = mod
        antenv.axon_hooks = mod

        from trn_agent_boot.trn_boot import _ntff_profile_via_ctypes

        hook = _ntff_profile_via_ctypes("/opt/axon/libaxon_pjrt.so")
        if hook is not None:
            set_axon_ntff_profile_hook(hook)
    except Exception:
        pass


def _build():
    import concourse.bacc as bacc
    import concourse.mybir as mybir
    import concourse.tile as tile

    f32 = mybir.dt.float32
    f32r = mybir.dt.float32r

    nc = bacc.Bacc(
        "TRN2", target_bir_lowering=False, debug=False, num_devices=N_CORES
    )

    xt = [
        nc.dram_tensor(f"xt{l}", [CH, ROWS[l]], f32r, kind="ExternalInput").ap()
        for l in range(LMAX + 1)
    ]
    w = nc.dram_tensor("w", [(LMAX + 1) * CH, CH], f32r, kind="ExternalInput").ap()
    pwb = nc.dram_tensor("pwb", [CH, LMAX + 1], f32, kind="ExternalInput").ap()
    outT = [
        nc.dram_tensor(f"outT{l}", [CH, ROWS[l]], f32, kind="ExternalOutput").ap()
        for l in range(LMAX + 1)
    ]

    with tile.TileContext(nc) as tc:
        with (
            tc.tile_pool(name="const", bufs=1) as cpool,
            tc.tile_pool(name="io", bufs=4) as iopool,
            tc.tile_pool(name="psum", bufs=4, space="PSUM") as pspool,
        ):
            w_sb = cpool.tile([CH, LMAX + 1, CH], f32r)
            for l in range(LMAX + 1):
                nc.sync.dma_start(w_sb[:, l, :], w[l * CH : (l + 1) * CH, :])
            pw_sb = cpool.tile([CH, LMAX + 1], f32)
            nc.sync.dma_start(pw_sb[:, :], pwb[:, :])

            for l in range(LMAX + 1):
                for j0 in range(0, ROWS[l], CHUNK):
                    cw = min(CHUNK, ROWS[l] - j0)
                    xt_sb = iopool.tile([CH, CHUNK], f32r, tag="xt")
                    nc.sync.dma_start(xt_sb[:, :cw], xt[l][:, j0 : j0 + cw])
                    out_sb = iopool.tile([CH, CHUNK], f32, tag="out")
                    for k0 in range(0, cw, MM):
                        n = min(MM, cw - k0)
                        ps = pspool.tile([CH, MM], f32)
                        nc.tensor.matmul(
                            ps[:, :n],
                            w_sb[:, l, :],
                            xt_sb[:, k0 : k0 + n],
                            start=True,
                            stop=True,
                        )
                        nc.vector.tensor_scalar_mul(
                            out_sb[:, k0 : k0 + n], ps[:, :n], pw_sb[:, l : l + 1]
                        )
                    # Stores on the ACT HWDGE ring (separate logical DMA queue
                    # from the SP ring carrying the loads).
                    nc.scalar.dma_start(outT[l][:, j0 : j0 + cw], out_sb[:, :cw])

    nc.compile()
    return nc


def kernel(x, weights, path_weights):
    global _nc, LAST_RESULTS
    _install_ntff_hook()
    from concourse.bass_utils import run_bass_kernel_spmd

    if _nc is None:
        _nc = _build()

    x = np.asarray(x, dtype=np.float32)
    weights = np.asarray(weights, dtype=np.float32)
    path_weights = np.asarray(path_weights, dtype=np.float32)

    w_flat = np.ascontiguousarray(weights.reshape((LMAX + 1) * CH, CH))
    pwb = np.ascontiguousarray(
        np.broadcast_to(path_weights[None, :], (CH, LMAX + 1)), dtype=np.float32
    )

    in_maps = []
    for c in range(N_CORES):
        xc = x[c * NPC : (c + 1) * NPC]  # [NPC, 16, CH]
        m = {"w": w_flat, "pwb": pwb}
        for l in range(LMAX + 1):
            s, wd = l * l, 2 * l + 1
            m[f"xt{l}"] = np.ascontiguousarray(
                xc[:, s : s + wd, :].reshape(NPC * wd, CH).T
            )
        in_maps.append(m)

    res = run_bass_kernel_spmd(_nc, in_maps, core_ids=list(range(N_CORES)))
    LAST_RESULTS = res

    out = np.empty((N_NODES, (LMAX + 1) ** 2, CH), dtype=np.float32)
    for c in range(N_CORES):
        for l in range(LMAX + 1):
            s, wd = l * l, 2 * l + 1
            out[c * NPC : (c + 1) * NPC, s : s + wd, :] = (
                res.results[c][f"outT{l}"].T.reshape(NPC, wd, CH)
            )
    return out
